# revision 1
# baseline (speedup 1.0000x reference)
"""Trainium2 Bass kernel for the GNN message-passing module (nn_Att_60189671686752).

Strategy
--------
Edges are sorted by destination agent (hi) on the host and sharded across the
8 cores as contiguous *agent ranges* balanced by edge count.  Because every
edge of an agent lands on exactly one core, the per-agent scatter-add needs no
cross-core reduction at all.

Per core, sorted edges are cut into tiles of <=512 edges such that each tile's
agents form a window of <=128 consecutive agents, and consecutive windows are
disjoint and tile the core's agent range.  All activations are kept
feature-major [128 features x 512 edges] so every layer is a single
lhsT.T @ rhs matmul.  GroupNorm means are folded into centered weights;
variance is computed with a (1/128)-matrix matmul that yields the variance
already broadcast across partitions; 1/sqrt via ACT Sqrt + DVE
reciprocal_approx_fast.

The query branch (relu(gn(agts[hi] @ Wq.T)) @ W1b.T) is computed once per
agent (not per edge), written to DRAM, and each edge tile multiplies the
gathered 128-agent window by a 0/1 expansion matrix directly inside the
Wc1-accumulation matmul.  The scatter-add is a 0/1 segment matmul per tile
followed by an indirect row-scatter into a DRAM partial buffer (windows are
disjoint, so plain writes suffice).
"""

import sys

sys.path.insert(0, "/opt/trn_rl_repo")

import numpy as np
from contextlib import ExitStack

import concourse.bass as bass
import concourse.tile as tile
from concourse import bacc
from concourse import mybir
from concourse.bass import IndirectOffsetOnAxis
from concourse.bass_utils import run_bass_kernel_spmd
from concourse.masks import make_identity

AF = mybir.ActivationFunctionType
ALU = mybir.AluOpType
F32 = mybir.dt.float32
FR = mybir.dt.float32r
I32 = mybir.dt.int32

P = 128
TE = 512  # edge slots per tile
EPS = 1e-5
NCORES = 8
N_AGT = 50000
N_CTX = 100000
IO_BUFS = 5
ACT_BUFS = 6
ACT2_BUFS = 3
MM_BUFS = 4
VB_BUFS = 3
AUX_BUFS = 1


# ----------------------------------------------------------------------------
# host-side preparation
# ----------------------------------------------------------------------------

def _center(lhsT):
    """Fold GroupNorm mean-subtraction into the weights: subtract, for every
    input row, its mean over the output (M) dimension."""
    return (lhsT - lhsT.mean(axis=1, keepdims=True)).astype(np.float32)


def _plan_core(his, a_start, a_end):
    """Cut a core's sorted edge list into tiles.

    Returns list of (e0, ne, A0, na): edge slice [e0, e0+ne), agent window
    [A0, A0+na) with na<=128, ne<=TE; windows are disjoint and cover
    [a_start, a_end) exactly.
    """
    tiles = []
    ne_total = len(his)
    # group boundaries of equal agents
    if ne_total:
        starts = np.flatnonzero(np.r_[True, his[1:] != his[:-1]])
        ends = np.r_[starts[1:], ne_total]
        agents = his[starts]
    else:
        starts = ends = agents = np.array([], dtype=np.int64)

    cur_e0 = 0
    cur_A0 = a_start
    for g in range(len(starts)):
        a, gs, ge = int(agents[g]), int(starts[g]), int(ends[g])
        assert ge - gs <= TE, f"agent degree {ge - gs} > {TE}"
        # close current tile if adding this group would overflow edges/agents
        if (ge - cur_e0 > TE) or (a - cur_A0 >= P):
            na = min(a - cur_A0, P)
            tiles.append((cur_e0, gs - cur_e0, cur_A0, na))
            cur_e0 = gs
            cur_A0 += na
            # bridge any remaining agent gap with empty tiles
            while a - cur_A0 >= P:
                tiles.append((cur_e0, 0, cur_A0, P))
                cur_A0 += P
    # close final tile(s)
    while True:
        na = min(a_end - cur_A0, P)
        tiles.append((cur_e0, ne_total - cur_e0, cur_A0, na))
        cur_e0 = ne_total
        cur_A0 += na
        if cur_A0 >= a_end:
            break
    return tiles


def _prepare(agts, ctx, agt_ctrs, ctx_ctrs, hi, wi):
    E = hi.shape[0]
    order = np.argsort(hi, kind="stable")
    his_all = hi[order]
    wis_all = wi[order]

    # shard edges into 8 contiguous chunks cut at agent boundaries
    cuts = [0]
    for c in range(1, NCORES):
        p = c * E // NCORES
        while p < E and his_all[p] == his_all[p - 1]:
            p += 1
        cuts.append(p)
    cuts.append(E)

    a_bounds = [0]
    for c in range(1, NCORES):
        p = cuts[c]
        a_bounds.append(int(his_all[p]) if p < E else N_AGT)
    a_bounds.append(N_AGT)

    cores = []
    for c in range(NCORES):
        e0, e1 = cuts[c], cuts[c + 1]
        cores.append(
            dict(
                his=his_all[e0:e1],
                wis=wis_all[e0:e1],
                a_start=a_bounds[c],
                a_end=a_bounds[c + 1],
            )
        )

    plans = [
        _plan_core(co["his"], co["a_start"], co["a_end"]) for co in cores
    ]
    nT = max(len(p) for p in plans)
    nA_max = max(co["a_end"] - co["a_start"] for co in cores)
    nAC = (nA_max + TE - 1) // TE
    napad = nAC * TE

    dd_all = (agt_ctrs[his_all] - ctx_ctrs[wis_all]).astype(np.float32)  # [E,2]

    in_maps = []
    for c, (co, plan) in enumerate(zip(cores, plans)):
        his, wis = co["his"], co["wis"]
        a_start = co["a_start"]
        e_base = cuts[c]
        n_real = len(plan)

        e0s = np.array([t[0] for t in plan], dtype=np.int64)
        nes = np.array([t[1] for t in plan], dtype=np.int64)
        A0s = np.array([t[2] for t in plan], dtype=np.int64)
        nas = np.array([t[3] for t in plan], dtype=np.int64)

        # per real edge: tile index and slot within tile
        tidx = np.repeat(np.arange(n_real), nes)
        j = np.arange(len(his)) - np.repeat(e0s, nes)
        loc = his - np.repeat(A0s, nes)  # 0..127 local agent column

        dd = np.zeros((3, nT * TE), np.float32)
        slot = tidx * TE + j
        dd[0, slot] = dd_all[e_base:e_base + len(his), 0]
        dd[1, slot] = dd_all[e_base:e_base + len(his), 1]
        dd[2, slot] = 1.0

        ctxg = np.zeros((P, nT * TE), np.float32)
        ctxg[:, slot] = ctx[wis].T

        sseg = np.zeros((nT, P, TE), np.float32)
        sseg[tidx, j % P, (j // P) * P + loc] = 1.0

        sqt = np.zeros((nT, P, TE), np.float32)
        sqt[tidx, loc, j] = 1.0

        widx = np.empty((nT, P), np.int32)
        jj = np.arange(P)[None, :]
        widx[:n_real] = (A0s[:, None] - a_start) + jj
        trash = napad + jj
        widx[:n_real] = np.where(jj < nas[:, None], widx[:n_real], trash)
        widx[n_real:] = trash  # pad tiles -> all trash rows

        nA = co["a_end"] - a_start
        agtsT = np.zeros((P, napad), np.float32)
        agtsT[:, :nA] = agts[a_start:co["a_end"]].T

        in_maps.append(
            dict(dd=dd, ctxg=ctxg, sseg=sseg, sqt=sqt, widx=widx[:, :, None],
                 agtsT=agtsT)
        )

    meta = dict(nT=nT, nAC=nAC, napad=napad,
                a_bounds=a_bounds)
    return in_maps, meta


def _prep_weights(Wd1, bd1, Wd2, Wq, Wc1, Wc2, Wa, Wl):
    w = {}
    w["wd1aug"] = np.concatenate(
        [Wd1.T.astype(np.float32), bd1[None, :].astype(np.float32)], axis=0
    )  # [3,128]
    w["wd2c"] = _center(Wd2.T)
    w["wqc"] = _center(Wq.T)
    w["w1a"] = _center(Wc1[:, 0:P].T)
    w["w1b"] = _center(Wc1[:, P:2 * P].T)
    w["w1c"] = _center(Wc1[:, 2 * P:3 * P].T)
    w["wc2r"] = Wc2.T.astype(np.float32).copy()  # rhs form [g, f]
    w["wa"] = Wa.T.astype(np.float32).copy()
    w["wlc"] = _center(Wl.T)
    w["umat"] = np.full((P, P), 1.0 / P, np.float32)
    w["identm"] = np.eye(P, dtype=np.float32)
    w["zerom"] = np.zeros((P, P), np.float32)
    return w


# ----------------------------------------------------------------------------
# device program
# ----------------------------------------------------------------------------

def _build(nT, nAC, napad, stages='ABC', fastgn=False):
    nc = bacc.Bacc(None, target_bir_lowering=False, debug=False)

    tw = {}
    for name, shape in [
        ("wd1aug", (3, P)), ("wd2c", (P, P)), ("wqc", (P, P)),
        ("w1a", (P, P)), ("w1b", (P, P)), ("w1c", (P, P)),
        ("wc2r", (P, P)), ("wa", (P, P)), ("wlc", (P, P)),
        ("umat", (P, P)), ("identm", (P, P)), ("zerom", (P, P)),
    ]:
        tw[name] = nc.dram_tensor(name, shape, FR, kind="ExternalInput")
    t_gv = nc.dram_tensor("gv", (P, 10), F32, kind="ExternalInput")

    t_dd = nc.dram_tensor("dd", (3, nT * TE), FR, kind="ExternalInput")
    t_ctx = nc.dram_tensor("ctxg", (P, nT * TE), FR, kind="ExternalInput")
    t_sseg = nc.dram_tensor("sseg", (nT, P, TE), FR, kind="ExternalInput")
    t_sqt = nc.dram_tensor("sqt", (nT, P, TE), FR, kind="ExternalInput")
    t_widx = nc.dram_tensor("widx", (nT, P, 1), I32, kind="ExternalInput")
    t_agts = nc.dram_tensor("agtsT", (P, napad), FR, kind="ExternalInput")

    t_qb = nc.dram_tensor("qbt", (napad + P, P), FR, kind="ExternalOutput")
    t_part = nc.dram_tensor("partial", (napad + P, P), FR,
                            kind="ExternalOutput")
    t_out = nc.dram_tensor("out", (P, napad), FR, kind="ExternalOutput")

    with tile.TileContext(nc) as tc, ExitStack() as ctx:
        const = ctx.enter_context(tc.tile_pool(name="const", bufs=1))
        io = ctx.enter_context(tc.tile_pool(name="io", bufs=IO_BUFS))
        act = ctx.enter_context(tc.tile_pool(name="act", bufs=ACT_BUFS))
        act2 = ctx.enter_context(tc.tile_pool(name="act2", bufs=ACT2_BUFS))
        ps = ctx.enter_context(tc.tile_pool(name="ps", bufs=MM_BUFS, space="PSUM"))
        psx = ctx.enter_context(tc.tile_pool(name="psx", bufs=VB_BUFS, space="PSUM"))
        psa = ctx.enter_context(tc.tile_pool(name="psa", bufs=AUX_BUFS, space="PSUM"))

        W = {}
        for name, handle in tw.items():
            t = const.tile(list(handle.shape), FR, tag=name)
            nc.sync.dma_start(t[:], handle[:, :])
            W[name] = t
        gv = const.tile([P, 10], F32, tag="gv")
        nc.sync.dma_start(gv[:], t_gv[:, :])
        gd2w, gd2b = gv[:, 0:1], gv[:, 1:2]
        gqw, gqb = gv[:, 2:3], gv[:, 3:4]
        gc1w, gc1b = gv[:, 4:5], gv[:, 5:6]
        gnw, gnb = gv[:, 6:7], gv[:, 7:8]
        glw, glb = gv[:, 8:9], gv[:, 9:10]

        ident = W["identm"][:]
        zero_b = const.tile([P, 1], F32, tag="zero_b")
        nc.gpsimd.memset(zero_b[:], 0.0)
        # pre-zero DRAM scratch regions the program reads but may not write
        nc.sync.dma_start(t_qb[napad:napad + P, :], W["zerom"][:])
        for r in range(0, napad + P, P):
            nc.sync.dma_start(t_part[r:r + P, :], W["zerom"][:])
        eps_b = const.tile([P, 1], F32, tag="eps_b")
        nc.gpsimd.memset(eps_b[:], EPS)

        def gn_apply(z_psum, w_ap, b_ap, n, with_mean=False, relu=True,
                     src_sbuf=False):
            """z_psum: [P, n] PSUM (or SBUF if src_sbuf), pre-centered unless
            with_mean.  Returns SBUF tile [P, n]:
            relu((z - mu) * rsqrt(var+eps) * w + b)
            (or the un-affined normalized value if relu=False)."""
            if with_mean:
                if src_sbuf:
                    zs = z_psum
                else:
                    zs_t = act2.tile([P, n], FR, tag="gn_zs")
                    nc.scalar.activation(zs_t[:], z_psum, AF.Copy)
                    zs = zs_t[:]
                mb = psx.tile([P, n], F32, tag="gn_vb")
                nc.tensor.matmul(mb[:], W["umat"][:], zs,
                                 start=True, stop=True)
                src = act2.tile([P, n], F32, tag="gn_zc")
                nc.vector.tensor_tensor(src[:], zs, mb[:],
                                        op=ALU.subtract)
                src = src[:]
            else:
                src = z_psum
            sq = act.tile([P, n], FR, tag="gn_sq")
            nc.scalar.activation(sq[:], src, AF.Square, bias=zero_b[:])
            vb = psx.tile([P, n], F32, tag="gn_vb")
            nc.tensor.matmul(vb[:], W["umat"][:], sq[:], start=True, stop=True)
            sd = act.tile([P, n], F32, tag="gn_sd")
            nc.scalar.activation(sd[:], vb[:], AF.Sqrt, bias=eps_b[:])
            rs = act.tile([P, n], F32, tag="gn_rs")
            nc.vector.reciprocal_approx_fast(out=rs[:], in_=sd[:])
            if relu and fastgn:
                # w==1, b==0: relu(z*rs) == relu(z)*rs; relu overlaps the
                # stats chain and frees the PSUM source earlier
                hp = act.tile([P, n], FR, tag="gn_tm")
                nc.scalar.activation(hp[:], src, AF.Relu, bias=zero_b[:])
                out = act.tile([P, n], FR, tag="gn_out")
                nc.vector.tensor_tensor(out[:], hp[:], rs[:], op=ALU.mult)
                return out
            tm = act.tile([P, n], F32, tag="gn_tm")
            nc.vector.tensor_tensor(tm[:], src, rs[:], op=ALU.mult)
            if not relu:
                return tm
            out = act.tile([P, n], FR, tag="gn_out")
            nc.scalar.activation(out[:], tm[:], AF.Relu,
                                 scale=w_ap, bias=b_ap)
            return out

        def gn_pre(z_psum, n, sname, want_hp=True):
            """Emit square + (fastgn) early relu + stat-broadcast matmul."""
            g = {"z": z_psum}
            g["sq"] = act.tile([P, n], FR, tag="gn_sq", name=f"sq{sname}")
            nc.scalar.activation(g["sq"][:], z_psum, AF.Square, bias=zero_b[:])
            if fastgn and want_hp:
                g["hp"] = act.tile([P, n], FR, tag="gn_tm", name=f"hp{sname}")
                nc.scalar.activation(g["hp"][:], z_psum, AF.Relu,
                                     bias=zero_b[:])
            g["vb"] = psx.tile([P, n], F32, tag="gn_vb", name=f"vb{sname}")
            nc.tensor.matmul(g["vb"][:], W["umat"][:], g["sq"][:],
                             start=True, stop=True)
            return g

        def gn_post(g, w_ap, b_ap, n, sname):
            sd = act.tile([P, n], F32, tag="gn_sd", name=f"sd{sname}")
            nc.scalar.activation(sd[:], g["vb"][:], AF.Sqrt, bias=eps_b[:])
            rs = act.tile([P, n], F32, tag="gn_rs", name=f"rs{sname}")
            nc.vector.reciprocal_approx_fast(out=rs[:], in_=sd[:])
            out = act.tile([P, n], FR, tag="gn_out", name=f"gno{sname}")
            if fastgn:
                nc.vector.tensor_tensor(out[:], g["hp"][:], rs[:],
                                        op=ALU.mult)
                return out
            tm = act.tile([P, n], F32, tag="gn_tm", name=f"tm{sname}")
            nc.vector.tensor_tensor(tm[:], g["z"], rs[:], op=ALU.mult)
            nc.scalar.activation(out[:], tm[:], AF.Relu,
                                 scale=w_ap, bias=b_ap)
            return out

        def load_fm(dram, row0):
            """Load TE rows [row0, row0+TE) of a [*, P] DRAM tensor and
            transpose into a feature-major [P, TE] SBUF tile."""
            fm = act2.tile([P, TE], FR, tag="fm")
            for k in range(4):
                t_in = io.tile([P, P], FR, tag="ld_am")
                nc.sync.dma_start(t_in[:],
                                  dram[row0 + k * P: row0 + (k + 1) * P, :])
                tp = psa.tile([P, P], FR, tag="aux")
                nc.tensor.transpose(tp[:], t_in[:], ident)
                nc.scalar.activation(fm[:, k * P:(k + 1) * P], tp[:], AF.Copy)
            return fm

        def store_am(dram, row0, fm_sbuf, dt_out):
            """Transpose a feature-major [P, TE] SBUF tile to agent-major and
            store to TE rows of a [*, P] DRAM tensor."""
            for k in range(4):
                tp = psa.tile([P, P], fm_sbuf.dtype, tag="aux")
                nc.tensor.transpose(tp[:], fm_sbuf[:, k * P:(k + 1) * P],
                                    ident)
                ob = act2.tile([P, P], dt_out, tag="st_am")
                nc.vector.tensor_copy(ob[:], tp[:])
                nc.sync.dma_start(dram[row0 + k * P: row0 + (k + 1) * P, :],
                                  ob[:])

        # ---- stage A: per-agent query branch -> QB table (pipelined) ----
        nA_ = nAC if 'A' in stages else 0
        sa_state = {}

        def a_g0(ch):
            s = {}
            s["fm"] = act2.tile([P, TE], FR, tag="fm", name=f"agfm{ch}")
            nc.sync.dma_start(s["fm"][:], t_agts[:, ch * TE:(ch + 1) * TE])
            s["zq"] = ps.tile([P, TE], F32, tag="mm", name=f"zq{ch}")
            nc.tensor.matmul(s["zq"][:], W["wqc"][:], s["fm"][:],
                             start=True, stop=True)
            s["g"] = gn_pre(s["zq"][:], TE, f"q{ch}")
            sa_state[ch] = s

        def a_g1(ch):
            s = sa_state.pop(ch)
            q = gn_post(s["g"], gqw, gqb, TE, f"q{ch}")
            qb = ps.tile([P, TE], F32, tag="mm", name=f"qb{ch}")
            nc.tensor.matmul(qb[:], W["w1b"][:], q[:], start=True, stop=True)
            qbs = act2.tile([P, TE], FR, tag="qbs", name=f"qbs{ch}")
            nc.scalar.activation(qbs[:], qb[:], AF.Copy)
            store_am(t_qb, ch * TE, qbs, FR)

        for i in range(nA_ + 1):
            if i < nA_:
                a_g0(i)
            if i >= 1:
                a_g1(i - 1)

        # ---- stage B: edge tiles (5-stage software pipeline) ----
        # Engines run their instruction streams in order, so cross-tile
        # overlap is expressed by interleaving the emitted program.
        nB = nT if 'B' in stages else 0
        st_state = {}

        def b_g0(t):
            s = {}
            s["dd"] = io.tile([3, TE], FR, tag="dd", name=f"dd{t}")
            nc.sync.dma_start(s["dd"][:], t_dd[:, t * TE:(t + 1) * TE])
            s["ctxt"] = io.tile([P, TE], FR, tag="ctx", name=f"ctxt{t}")
            nc.sync.dma_start(s["ctxt"][:], t_ctx[:, t * TE:(t + 1) * TE])
            s["ss"] = io.tile([P, TE], FR, tag="ss", name=f"ss{t}")
            nc.gpsimd.dma_start(s["ss"][:], t_sseg[t])
            s["st"] = io.tile([P, TE], FR, tag="st", name=f"sqt{t}")
            nc.gpsimd.dma_start(s["st"][:], t_sqt[t])
            s["wix"] = io.tile([P, 1], I32, tag="wix", name=f"wix{t}")
            nc.sync.dma_start(s["wix"][:], t_widx[t])
            s["qwin"] = io.tile([P, P], FR, tag="qwin", name=f"qwin{t}")
            nc.gpsimd.indirect_dma_start(
                out=s["qwin"][:], out_offset=None, in_=t_qb[:, :],
                in_offset=IndirectOffsetOnAxis(ap=s["wix"][:, :1], axis=0),
            )
            h1p = ps.tile([P, TE], F32, tag="mm", name=f"h1p{t}")
            nc.tensor.matmul(h1p[:], W["wd1aug"][:], s["dd"][:],
                             start=True, stop=True)
            s["h1"] = act.tile([P, TE], FR, tag="h1", name=f"h1{t}")
            nc.vector.tensor_scalar(s["h1"][:], h1p[:], 0.0, None,
                                    op0=ALU.max)
            s["z2"] = ps.tile([P, TE], F32, tag="mm", name=f"z2{t}")
            nc.tensor.matmul(s["z2"][:], W["wd2c"][:], s["h1"][:],
                             start=True, stop=True)
            st_state[t] = s

        def b_g1a(t):
            s = st_state[t]
            s["g2"] = gn_pre(s["z2"][:], TE, f"z{t}")

        def b_g1b(t):
            s = st_state[t]
            h2 = gn_post(s["g2"], gd2w, gd2b, TE, f"z{t}")
            c1 = ps.tile([P, TE], F32, tag="mm", name=f"c1{t}")
            nc.tensor.matmul(c1[:], W["w1a"][:], h2[:],
                             start=True, stop=False)
            nc.tensor.matmul(c1[:], W["w1c"][:], s["ctxt"][:],
                             start=False, stop=False)
            nc.tensor.matmul(c1[:], s["qwin"][:], s["st"][:],
                             start=False, stop=True)
            s["c1"] = c1

        def b_g2a(t):
            s = st_state[t]
            s["gc"] = gn_pre(s["c1"][:], TE, f"c{t}")

        def b_g2b(t):
            s = st_state.pop(t)
            c = gn_post(s["gc"], gc1w, gc1b, TE, f"c{t}")
            me = ps.tile([P, TE], F32, tag="mm", name=f"me{t}")
            for k in range(4):
                nc.tensor.matmul(me[:, k * P:(k + 1) * P],
                                 c[:, k * P:(k + 1) * P], W["wc2r"][:],
                                 start=True, stop=True)
            mes = act.tile([P, TE], FR, tag="mes", name=f"mes{t}")
            nc.vector.tensor_copy(mes[:], me[:])
            segp = psx.tile([P, P], F32, tag="gn_vb", name=f"segp{t}")
            for k in range(4):
                nc.tensor.matmul(segp[:], s["ss"][:, k * P:(k + 1) * P],
                                 mes[:, k * P:(k + 1) * P],
                                 start=(k == 0), stop=(k == 3))
            segs = act.tile([P, P], FR, tag="segs", name=f"segs{t}")
            nc.vector.tensor_copy(segs[:], segp[:])
            nc.gpsimd.indirect_dma_start(
                out=t_part[:, :],
                out_offset=IndirectOffsetOnAxis(ap=s["wix"][:, :1], axis=0),
                in_=segs[:], in_offset=None,
            )

        phases = [b_g0, b_g1a, b_g1b, b_g2a, b_g2b]
        for i in range(nB + len(phases) - 1):
            for d, ph in enumerate(phases):
                t = i - d
                if 0 <= t < nB:
                    ph(t)

        # ---- stage C: per-agent tail (pipelined) ----
        nC_ = nAC if 'C' in stages else 0
        sc_state = {}

        def c_g0(ch):
            s = {}
            s["fm"] = act2.tile([P, TE], FR, tag="fm", name=f"cfm{ch}")
            nc.sync.dma_start(s["fm"][:], t_agts[:, ch * TE:(ch + 1) * TE])
            apz = ps.tile([P, TE], F32, tag="mm", name=f"apz{ch}")
            nc.tensor.matmul(apz[:], W["wa"][:], s["fm"][:],
                             start=True, stop=True)
            pfm = act2.tile([P, TE], F32, tag="pfm", name=f"pfm{ch}")
            for k in range(4):
                pin = io.tile([P, P], FR, tag="pin", name=f"pin{ch}_{k}")
                nc.sync.dma_start(
                    pin[:],
                    t_part[ch * TE + k * P: ch * TE + (k + 1) * P, :])
                tp = psa.tile([P, P], FR, tag="aux", name=f"ctp{ch}_{k}")
                nc.tensor.transpose(tp[:], pin[:], ident)
                nc.scalar.activation(pfm[:, k * P:(k + 1) * P], tp[:],
                                     AF.Copy)
            a_sb = act2.tile([P, TE], FR, tag="gn_zs", name=f"asb{ch}")
            nc.vector.tensor_tensor(a_sb[:], pfm[:], apz[:], op=ALU.add)
            s["a_sb"] = a_sb
            del s["fm"]
            sc_state[ch] = s

        def c_g1(ch):
            s = sc_state[ch]
            zs = s["a_sb"][:]
            mb = psx.tile([P, TE], F32, tag="gn_vb", name=f"mb{ch}")
            nc.tensor.matmul(mb[:], W["umat"][:], zs, start=True, stop=True)
            zc = act2.tile([P, TE], FR, tag="gn_zc", name=f"zc{ch}")
            nc.vector.tensor_tensor(zc[:], zs, mb[:], op=ALU.subtract)
            s["gm"] = gn_pre(zc[:], TE, f"a{ch}")
            s["zc"] = zc

        def c_g2(ch):
            s = sc_state[ch]
            g = s["gm"]
            n = TE
            sname = f"a{ch}"
            sd = act.tile([P, n], F32, tag="gn_sd", name=f"sd{sname}")
            nc.scalar.activation(sd[:], g["vb"][:], AF.Sqrt, bias=eps_b[:])
            rs = act.tile([P, n], F32, tag="gn_rs", name=f"rs{sname}")
            nc.vector.reciprocal_approx_fast(out=rs[:], in_=sd[:])
            an = act.tile([P, n], FR, tag="gn_out", name=f"an{ch}")
            if fastgn:
                nc.vector.tensor_tensor(an[:], g["hp"][:], rs[:],
                                        op=ALU.mult)
            else:
                tm = act.tile([P, n], F32, tag="gn_tm", name=f"tmn{ch}")
                nc.vector.tensor_tensor(tm[:], s["zc"][:], rs[:],
                                        op=ALU.mult)
                nc.scalar.activation(an[:], tm[:], AF.Relu,
                                     scale=gnw, bias=gnb)
            zl = ps.tile([P, TE], F32, tag="mm", name=f"zl{ch}")
            nc.tensor.matmul(zl[:], W["wlc"][:], an[:], start=True, stop=True)
            s["gl"] = gn_pre(zl[:], TE, f"l{ch}", want_hp=False)
            s["zl"] = zl
            s["res"] = act2.tile([P, TE], FR, tag="res", name=f"res{ch}")
            nc.sync.dma_start(s["res"][:], t_agts[:, ch * TE:(ch + 1) * TE])

        def c_g3(ch):
            s = sc_state.pop(ch)
            g = s["gl"]
            n = TE
            sd = act.tile([P, n], F32, tag="gn_sd", name=f"sdl{ch}")
            nc.scalar.activation(sd[:], g["vb"][:], AF.Sqrt, bias=eps_b[:])
            rs = act.tile([P, n], F32, tag="gn_rs", name=f"rsl{ch}")
            nc.vector.reciprocal_approx_fast(out=rs[:], in_=sd[:])
            tl = act.tile([P, n], F32, tag="gn_tm", name=f"tll{ch}")
            nc.vector.tensor_tensor(tl[:], s["zl"][:], rs[:], op=ALU.mult)
            t2 = act2.tile([P, TE], F32, tag="fin2", name=f"t2{ch}")
            nc.vector.tensor_scalar(t2[:], tl[:], glw, glb,
                                    op0=ALU.mult, op1=ALU.add)
            t3 = act2.tile([P, TE], F32, tag="fin3", name=f"t3{ch}")
            nc.vector.tensor_tensor(t3[:], t2[:], s["res"][:], op=ALU.add)
            oc = act2.tile([P, TE], FR, tag="oc", name=f"oc{ch}")
            nc.scalar.activation(oc[:], t3[:], AF.Relu, bias=zero_b[:])
            nc.sync.dma_start(t_out[:, ch * TE:(ch + 1) * TE], oc[:])

        cphases = [c_g0, c_g1, c_g2, c_g3]
        for i in range(nC_ + len(cphases) - 1):
            for d, ph in enumerate(cphases):
                t = i - d
                if 0 <= t < nC_:
                    ph(t)

    nc.compile()
    return nc


_CACHE = {}


def kernel(agts, ctx, agt_ctrs, ctx_ctrs, hi, wi,
           Wd1, bd1, Wd2, gd2w, gd2b, Wq, gqw, gqb,
           Wc1, gc1w, gc1b, Wc2, Wa, gnw, gnb, Wl, glw, glb,
           _trace=False):
    agts = np.asarray(agts, np.float32)
    ctx = np.asarray(ctx, np.float32)
    agt_ctrs = np.asarray(agt_ctrs, np.float32)
    ctx_ctrs = np.asarray(ctx_ctrs, np.float32)
    hi = np.asarray(hi, np.int32)
    wi = np.asarray(wi, np.int32)

    in_maps, meta = _prepare(agts, ctx, agt_ctrs, ctx_ctrs, hi, wi)
    w = _prep_weights(np.asarray(Wd1, np.float32), np.asarray(bd1, np.float32),
                      np.asarray(Wd2, np.float32), np.asarray(Wq, np.float32),
                      np.asarray(Wc1, np.float32), np.asarray(Wc2, np.float32),
                      np.asarray(Wa, np.float32), np.asarray(Wl, np.float32))
    gvec = np.stack([np.asarray(v, np.float32) for v in
                     [gd2w, gd2b, gqw, gqb, gc1w, gc1b, gnw, gnb, glw, glb]],
                    axis=1)  # [128, 10]

    fastgn = all(
        np.all(np.asarray(wv, np.float32) == 1.0)
        and np.all(np.asarray(bv, np.float32) == 0.0)
        for wv, bv in [(gd2w, gd2b), (gqw, gqb), (gc1w, gc1b), (gnw, gnb)]
    )
    key = (meta["nT"], meta["nAC"], meta["napad"], fastgn)
    if key not in _CACHE:
        _CACHE[key] = _build(key[0], key[1], key[2], fastgn=key[3])
    nc = _CACHE[key]

    full_maps = []
    for m in in_maps:
        fm = dict(m)
        fm.update({k: w[k] for k in w})
        fm["gv"] = gvec
        full_maps.append(fm)

    try:
        res = run_bass_kernel_spmd(nc, full_maps,
                                   core_ids=list(range(NCORES)),
                                   trace=_trace)
    except ModuleNotFoundError:
        res = run_bass_kernel_spmd(nc, full_maps,
                                   core_ids=list(range(NCORES)),
                                   trace=False)

    out = np.empty((N_AGT, P), np.float32)
    ab = meta["a_bounds"]
    for c in range(NCORES):
        nA = ab[c + 1] - ab[c]
        out[ab[c]:ab[c + 1]] = res.results[c]["out"][:, :nA].T
    if _trace:
        kernel._last_exec_time_ns = res.exec_time_ns
        kernel._last_results = res
    return out

